# revision 41
# baseline (speedup 1.0000x reference)
"""Trainium2 Bass kernel for nn_MultiHeadAttention (B=2, S=2048, D=1024, H=16).

Sharding: 8 cores = 2 batch groups x 4 head-groups (4 heads/core).
Per core: QKV projections (fp8 DoubleRow for Q/K, bf16 for V), RoPE via
batched DVE mults + a PE permutation matmul that reuses the projection
psum in place (merged rotate-half layout: each score block is a single
64-contraction matmul), causal attention with explicitly row-tiled score
matmuls (two 64-row PE tiles run concurrently), bf16 PV with fused rowsum
via [V|ones] stationary, bf16 output projection partials; host sums the
4 partials per batch and adds the bias.

Self-contained: hardcodes shapes; only imports concourse/numpy.
"""
import numpy as np
from contextlib import ExitStack

import concourse.bass as bass
import concourse.bacc as bacc
import concourse.mybir as mybir
import concourse.tile as tile
from concourse.bass_utils import run_bass_kernel_spmd

F32 = mybir.dt.float32
BF16 = mybir.dt.bfloat16
I16 = mybir.dt.int16
AF = mybir.ActivationFunctionType
OP = mybir.AluOpType
NP_BF16 = mybir.dt.np(mybir.dt.bfloat16)

# Schraudolph-style exp on DVE: bf16 bits of exp(s/8) ~= round(EK*s + EB)
# (bf16 exponent step = 128 mantissa codes; c=0.0573 balances the error)
EK = 128.0 * 1.4426950408889634 * 0.125
EB = 128.0 * (127.0 - 0.0573)
DVE_EXP_MOD = 5   # every Nth (jb, hp) slot's exp runs on DVE, 0 = off
PV_PENDING = 4    # PV deferral depth (slots)
PT_BUFS = 6       # pt tile double-buffer depth
OST_ON_ACT = True   # all wo psum->sbuf copies on ACT (else odd dt on DVE)
VCOPY_ON_ACT = True  # V psum->sbuf copy on ACT
RT_ON_ACT = False    # rowsum psum->sbuf copy on ACT
FILL_LATE = False    # drain attention fillers only in the itile 2nd half

F8 = mybir.dt.float8e4
NP_F8 = mybir.dt.np(mybir.dt.float8e4)
SX, SW = 32.0, 512.0      # fp8 pre-scales for x and Wq/Wk (powers of 2)

B, S, D, H = 2, 2048, 1024, 16
HD = 64          # head dim
HPC = 4          # heads per core
N_CORES = 8
SC = 512         # s-chunk for projections / i-tile width
NSC = S // SC    # 4
NDB = D // 128   # 8 d-blocks
NJB = S // 128   # 16 j-blocks
DLOC = HPC * HD  # 256 local channels


def build_program(niter=1, phases='paw', dbg=False, unroll=1):
    """Build + compile the per-core SPMD program. niter>1 wraps the whole
    kernel in a device-side loop (for timing)."""
    nc = bacc.Bacc("TRN2", target_bir_lowering=False, debug=False,
                   num_devices=N_CORES)

    # host-pre-tiled layouts: flat per-partition strips -> few descriptors
    d_xt = nc.dram_tensor("xt", [128, NSC, NDB, SC], BF16,
                          kind="ExternalInput").ap()
    d_xt8 = nc.dram_tensor("xt8", [128, NSC, NDB, SC], F8,
                           kind="ExternalInput").ap()
    d_wq = nc.dram_tensor("wq", [128, NDB, DLOC], F8,
                          kind="ExternalInput").ap()
    d_wk = nc.dram_tensor("wk", [128, NDB, DLOC], F8,
                          kind="ExternalInput").ap()
    d_wv = nc.dram_tensor("wv", [128, NDB, DLOC], BF16,
                          kind="ExternalInput").ap()
    d_wo = nc.dram_tensor("wo", [128, 2, D], BF16, kind="ExternalInput").ap()
    d_cos = nc.dram_tensor("cosT", [128, S], BF16, kind="ExternalInput").ap()
    d_sin = nc.dram_tensor("sinT", [128, S], BF16, kind="ExternalInput").ap()
    d_pm = nc.dram_tensor("pmat", [128, 128], BF16, kind="ExternalInput").ap()
    d_ones = nc.dram_tensor("ones", [128, NJB * HPC * 64], BF16,
                            kind="ExternalInput").ap()
    d_mask = nc.dram_tensor("masktri", [128, 2, 128], BF16,
                            kind="ExternalInput").ap()
    d_out = nc.dram_tensor("outp", [128, NSC, NDB, SC], BF16,
                           kind="ExternalOutput").ap()

    with tile.TileContext(nc) as tc:
        ctx = ExitStack()
        sb = ctx.enter_context(tc.tile_pool(name="sb", bufs=1))
        ps = ctx.enter_context(tc.tile_pool(name="ps", bufs=1, space="PSUM"))

        # persistent inputs (loaded once, outside the timing loop)
        wq = sb.tile([128, NDB, DLOC], F8, tag="wq")
        wk = sb.tile([128, NDB, DLOC], F8, tag="wk")
        wv = sb.tile([128, NDB, DLOC], BF16, tag="wv")
        wo = sb.tile([128, 2, D], BF16, tag="wo")
        cosT = sb.tile([128, S], BF16, tag="cs0")
        sinT = sb.tile([128, S], BF16, tag="cs1")
        pmat = sb.tile([128, 128], BF16, tag="pm")
        mtri2 = sb.tile([128, 2, 128], BF16, tag="mt2")
        # ordered by first use (Q/K weights + rope tables first — the QK
        # projection is the first PE work) and spread across DMA queues
        nc.gpsimd.dma_start(wq[:], d_wq[:])
        nc.gpsimd.dma_start(wk[:], d_wk[:])
        nc.scalar.dma_start(cosT[:], d_cos[:])
        nc.scalar.dma_start(sinT[:], d_sin[:])
        nc.scalar.dma_start(pmat[:], d_pm[:])
        nc.scalar.dma_start(mtri2[:], d_mask[:])
        nc.scalar.dma_start(wv[:], d_wv[:])
        # V' = [j, jb, head, V(64)|ones(64)] -- persistent; ones loaded once
        vp = sb.tile([128, NJB, HPC, 128], BF16, tag="vp")
        nc.scalar.dma_start(
            vp[:, :, :, 64:128],
            d_ones[:].rearrange("p (j h c) -> p j h c", j=NJB, h=HPC))
        nc.gpsimd.dma_start(wo[:], d_wo[:])

        # rope'd Q/K in merged rotate-half layout:
        # qf[p, hp, s]: half hp holds heads (2hp, 2hp+1); within a half,
        # head block rows = [32 rot-evens | 32 rot-odds].
        qf = sb.tile([128, 2, S], BF16, tag="qf")
        kf = sb.tile([128, 2, S], BF16, tag="kf")
        cxt = sb.tile([128, 2, S], BF16, tag="cx")  # ctxT, halves=head pairs
        # dedicated chunk-0 x tiles: their load is prefetched from the tail
        # of the previous loop iteration (chunks 1-3 double-buffer on a tag)
        xt0 = sb.tile([128, NDB, SC], BF16, tag="xt0")
        xt80 = sb.tile([128, NDB, SC], F8, tag="xt80", name="xt80")

        def load_xt0():
            nc.sync.dma_start(xt80[:], d_xt8[:, 0])
            nc.sync.dma_start(xt0[:], d_xt[:, 0])

        def body(_i=None, in_loop=False, only_proj0=False, carry=False):

            xt_tiles = {}
            xt8_tiles = {}
            if in_loop:
                # chunk 0 was loaded by the previous iteration (or prologue)
                xt_tiles[0] = xt0
                xt8_tiles[0] = xt80

            def xt_load_gen(sc):
                if sc == 0:
                    xt, xt8 = xt0, xt80
                else:
                    xt = sb.tile([128, NDB, SC], BF16, tag="xt", bufs=2,
                                 name="xt")
                    xt8 = sb.tile([128, NDB, SC], F8, tag="xt8", bufs=2,
                                  name="xt8")
                nc.sync.dma_start(xt8[:], d_xt8[:, sc])
                nc.sync.dma_start(xt[:], d_xt[:, sc])
                xt_tiles[sc] = xt
                xt8_tiles[sc] = xt8
                yield

            def proj_qk_gen(sc):
                """Q/K projection + RoPE for chunk `sc` as PE-work quanta.

                pp rides the shared 'st' rotation (with the score psum)
                so quanta can interleave into the attention loop; the swap
                matmul writes back into pp's banks once the rope mults
                have consumed them.
                """
                s0 = sc * SC
                if sc not in xt_tiles:
                    yield from xt_load_gen(sc)
                xt8 = xt8_tiles[sc]
                csb = cosT[:, s0:s0 + SC].unsqueeze(1).broadcast_to(
                    [128, 2, SC])
                snb = sinT[:, s0:s0 + SC].unsqueeze(1).broadcast_to(
                    [128, 2, SC])

                def proj_mm(w, pp, hp):
                    for t in range(NDB // 2):
                        nc.tensor.matmul(
                            pp[:, hp, :],
                            w[:, 2 * t:2 * t + 2, 128 * hp:128 * hp + 128],
                            xt8[:, 2 * t:2 * t + 2, :],
                            start=(t == 0), stop=(t == NDB // 2 - 1),
                            perf_mode=mybir.MatmulPerfMode.DoubleRow)

                def rope_mults(pp, t1, t2):
                    nc.vector.tensor_tensor(t1[:], pp[:], csb, OP.mult)
                    nc.vector.tensor_tensor(t2[:], pp[:], snb, OP.mult)

                def swap_mm(pp, t2):
                    # pp <- pmat @ t2 (reuses pp's own banks)
                    for hp in range(2):
                        nc.tensor.matmul(pp[:, hp, :], pmat[:],
                                         t2[:, hp, :], start=True, stop=True)

                def rope_add(dst, t1, pp):
                    nc.vector.tensor_tensor(dst[:, :, s0:s0 + SC],
                                            t1[:], pp[:], OP.add)

                def mk_tiles():
                    pp = ps.tile([128, 2, SC], F32, tag="st", bufs=2,
                                 name="pp")
                    t1 = sb.tile([128, 2, SC], BF16, tag="t1", bufs=2,
                                 name="t1")
                    t2 = sb.tile([128, 2, SC], BF16, tag="t2", bufs=2,
                                 name="t2")
                    return pp, t1, t2

                # software-pipelined: each swap runs >=2 slots after its
                # rope mults so the in-order PE never waits on the DVE
                ppq, t1q, t2q = mk_tiles()
                proj_mm(wq, ppq, 0)
                yield
                proj_mm(wq, ppq, 1)
                rope_mults(ppq, t1q, t2q)
                yield
                ppk, t1k, t2k = mk_tiles()
                proj_mm(wk, ppk, 0)
                yield
                swap_mm(ppq, t2q)
                yield
                proj_mm(wk, ppk, 1)
                rope_mults(ppk, t1k, t2k)
                rope_add(qf, t1q, ppq)
                yield
                yield
                swap_mm(ppk, t2k)
                yield
                rope_add(kf, t1k, ppk)
                yield

            def proj_v_gen(sc):
                """V projection for chunk `sc`; vps shares the pv banks so
                it may only run while no pv accumulation is live."""
                xt = xt_tiles[sc]
                for vb in range(2):
                    jb0 = sc * (SC // 128) + 2 * vb
                    vps = ps.tile([128, 2, DLOC], F32, tag=f"pv{2 * vb}",
                                  name="vps")
                    for k in range(2):
                        sb4 = 2 * vb + k
                        for db in range(NDB):
                            nc.tensor.matmul(
                                vps[:, k, :],
                                xt[:, db, 128 * sb4:128 * sb4 + 128],
                                wv[:, db, :],
                                start=(db == 0), stop=(db == NDB - 1))
                    if VCOPY_ON_ACT:
                        nc.scalar.activation(
                            vp[:, jb0:jb0 + 2, :, 0:64],
                            vps[:].rearrange("p b (h c) -> p b h c", h=HPC),
                            AF.Copy)
                    else:
                        nc.vector.tensor_copy(
                            vp[:, jb0:jb0 + 2, :, 0:64],
                            vps[:].rearrange("p b (h c) -> p b h c", h=HPC))
                    yield

            def wo_gen(it):
                """Output projection for i-tile `it` as PE-work quanta."""
                i0 = it * SC
                ost = sb.tile([128, NDB, SC], BF16, tag="ost", bufs=2,
                              name="ost")
                for dt in range(NDB):
                    ops_ = ps.tile([128, SC], F32, tag=f"pv{2 * (dt % 2)}",
                                   name="ops")
                    for dcb in range(2):
                        nc.tensor.matmul(
                            ops_[:], wo[:, dcb, dt * 128:dt * 128 + 128],
                            cxt[:, dcb, i0:i0 + SC],
                            start=(dcb == 0), stop=(dcb == 1))
                    if OST_ON_ACT or dt % 2 == 0:
                        nc.scalar.activation(ost[:, dt, :], ops_[:], AF.Copy)
                    else:
                        nc.vector.tensor_copy(ost[:, dt, :], ops_[:])
                    # stream the output out per d-block so the final DMA
                    # tail after the last i-tile is short
                    nc.sync.dma_start(d_out[:, it, dt], ost[:, dt, :])
                    yield

            def drain(gens):
                for g in gens:
                    for _ in g:
                        pass

            def attn_itile(it, fill=(), pre=(), post=()):
                """Attention i-tile; drains one filler quantum per
                (jb, head-pair) slot so the PE never idles on exp.
                `pre` generators drain fully at entry (before the pv
                accumulators are allocated); `post` after normalize."""
                drain(pre)

                def filler():
                    for g in fill:
                        yield from g
                f = filler()
                i0 = it * SC
                njb = i0 // 128 + 4
                # pvs2[g][:, u, :] accumulates head 2g+u; [64:128] = rowsum
                pvs2 = [ps.tile([128, 2, SC], F32, tag=f"pv{2 * g}",
                                name=f"pvt{g}") for g in range(2)]
                pending = []
                escale = 0.125
                slot = 0
                for jb in range(njb):
                    j0 = jb * 128
                    off = max(0, j0 - i0)
                    for hp in range(2):  # head pairs (0,1) and (2,3)
                        ss = ps.tile([128, 2, SC], F32, tag="st", bufs=2,
                                     name="ssjb")
                        for hh in range(2):
                            r0 = 64 * hh
                            nc.tensor.matmul(
                                ss[:, hh, off:SC],
                                kf[r0:r0 + 64, hp, j0:j0 + 128],
                                qf[r0:r0 + 64, hp, i0 + off:i0 + SC],
                                start=True, stop=True,
                                tile_position=(r0, 0))
                        pt = sb.tile([128, 2, SC], BF16, tag="pt",
                                     bufs=PT_BUFS, name="pt")
                        slot += 1
                        if DVE_EXP_MOD and slot % DVE_EXP_MOD == 0:
                            # approximate exp on DVE (bf16 bit trick) to
                            # take load off ACT, the attention-phase pacer
                            nc.vector.tensor_scalar(
                                pt[:, :, off:SC].bitcast(I16),
                                ss[:, :, off:SC], EK, EB, OP.mult, OP.add)
                        else:
                            nc.scalar.activation(pt[:, :, off:SC],
                                                 ss[:, :, off:SC],
                                                 AF.Exp, scale=escale)
                        if j0 >= i0:
                            # diagonal: mask both heads' [128,128] windows
                            nc.vector.tensor_tensor(
                                pt[:, :, off:off + 128],
                                pt[:, :, off:off + 128], mtri2[:], OP.mult)

                        def mk_pv(jb=jb, hp=hp, pt=pt, off=off):
                            for hh in range(2):
                                h = 2 * hp + hh
                                nc.tensor.matmul(
                                    pvs2[h // 2][:, h % 2, off:SC],
                                    vp[:, jb, h, :], pt[:, hh, off:SC],
                                    start=(jb == 0),
                                    stop=(jb == njb - 1))
                        pending.append(mk_pv)
                        # previous block's PV after this block's scores so
                        # the PE stream never stalls on the current exp
                        while len(pending) > PV_PENDING:
                            pending.pop(0)()
                        # drain filler only in the itile's second half: the
                        # proj psum (shared 'st' rotation) then dwells over
                        # the itile boundary instead of starving the scores
                        if not FILL_LATE or jb >= njb // 2:
                            next(f, None)
                for fn in pending:
                    fn()
                # normalize: ctx = pv[0:64] / r (r = pv[64:128])
                for g in range(2):
                    # reciprocal_approx_fast cannot read PSUM on HW --
                    # bounce the rowsums through SBUF first
                    rt = sb.tile([64, 2, SC], F32, tag="rt", bufs=2,
                                 name="rt")
                    if RT_ON_ACT:
                        nc.scalar.activation(rt[:], pvs2[g][64:128, :, :],
                                             AF.Copy)
                    else:
                        nc.vector.tensor_copy(rt[:], pvs2[g][64:128, :, :])
                    rr = sb.tile([64, 2, SC], F32, tag="rr", bufs=2,
                                 name="rr")
                    nc.vector.reciprocal_approx_fast(rr[:], rt[:])
                    for u in range(2):
                        h = 2 * g + u
                        nc.vector.tensor_tensor(
                            cxt[64 * (h % 2):64 * (h % 2) + 64, h // 2,
                                i0:i0 + SC],
                            pvs2[g][0:64, u, :], rr[:, u, :], OP.mult)
                drain([f])
                drain(post)

            if only_proj0:
                # prologue for the carried loop: chunk-0 proj only
                drain([proj_qk_gen(0), proj_v_gen(0)])
                return

            w_ = "w" in phases
            if "p" in phases and "a" in phases:
                if not carry:
                    drain([proj_qk_gen(0), proj_v_gen(0)])
                attn_itile(0, fill=[proj_qk_gen(1)], post=[proj_v_gen(1)])
                attn_itile(1, fill=[proj_qk_gen(2)], post=[proj_v_gen(2)],
                           pre=[wo_gen(0)] if w_ else ())
                attn_itile(2, fill=[proj_qk_gen(3)], post=[proj_v_gen(3)],
                           pre=[wo_gen(1)] if w_ else ())
                # carried loop: the NEXT iteration's chunk-0 reload + Q/K
                # projection fill attn3's exp-paced PE idle slots; its V
                # projection runs as post (pv banks)
                attn_itile(3, pre=[wo_gen(2)] if w_ else (),
                           fill=([xt_load_gen(0), proj_qk_gen(0)]
                                 if carry else ()),
                           post=([proj_v_gen(0)] if carry else ()))
                if w_:
                    drain([wo_gen(3)])
            elif "p" in phases:
                for sc_ in range(NSC):
                    drain([proj_qk_gen(sc_), proj_v_gen(sc_)])
            elif "a" in phases:
                for it_ in range(NSC):
                    attn_itile(it_)
                if w_:
                    for it_ in range(NSC):
                        drain([wo_gen(it_)])
            elif w_:
                for it_ in range(NSC):
                    drain([wo_gen(it_)])

        carried = "p" in phases and "a" in phases
        if unroll > 1:
            # unrolled steady-state emulation (no hw loop): body n times
            # with the same chunk-0 carry structure as the real loop
            load_xt0()
            if carried:
                body(in_loop=True, only_proj0=True)
            for _ in range(unroll):
                body(in_loop=carried, carry=carried)
        elif niter == 1:
            body()
        else:
            if "p" not in phases:
                # populate qf/kf/vp/cxt once so the timed loop has real data
                saved = phases
                phases = "pa"
                body()
                phases = saved
            load_xt0()  # prologue; in-loop reloads happen at attn3
            if carried:
                body(in_loop=True, only_proj0=True)
            with tc.For_i(0, niter, 1, staggered_reset=True) as i:
                body(i, in_loop=carried, carry=carried)
        ctx.close()
    nc.compile()
    return nc


def prep_inputs(x, pos_cos, pos_sin, Wq, Wk, Wv, Wo):
    """Host-side prep: per-core input dicts (pre-tiled)."""
    # fp8 scale folded out of the rope tables (tiny activation scales
    # underflow on HW, so fold into cos/sin instead)
    tscale = 1.0 / (SX * SW)
    cosT = np.ascontiguousarray(
        (np.tile(np.asarray(pos_cos, np.float32).T, (4, 1))
         * np.float32(tscale)).astype(NP_BF16))
    sinT = np.ascontiguousarray(
        (np.tile(np.asarray(pos_sin, np.float32).T, (4, 1))
         * np.float32(tscale)).astype(NP_BF16))
    ones = np.ones((128, NJB * HPC * 64), NP_BF16)
    # mask[j, i] keeps i >= j within the diagonal 128x128 window
    mtri = np.triu(np.ones((128, 128), np.float32)).astype(NP_BF16)
    mtri2 = np.ascontiguousarray(
        np.broadcast_to(mtri[:, None, :], (128, 2, 128)))
    # permutation matmul: out[p] = -t2[p+32] (even block) / +t2[p-32] (odd)
    pmat = np.zeros((128, 128), np.float32)
    for p in range(128):
        if (p % 64) < 32:
            pmat[p + 32, p] = -1.0
        else:
            pmat[p - 32, p] = 1.0

    def tile_rows(a, nb):
        # [nb*128, C] -> [128, nb, C]
        return np.ascontiguousarray(
            a.reshape(nb, 128, a.shape[1]).transpose(1, 0, 2))

    x = np.asarray(x, np.float32)
    Wq, Wk, Wv, Wo = (np.asarray(w, np.float32) for w in (Wq, Wk, Wv, Wo))
    in_maps = []
    for c in range(N_CORES):
        b, g = c // 4, c % 4
        heads = [4 * g + h for h in range(HPC)]
        # rotate-half head-major: per head [evens(32) | odds(32)]
        perm = np.concatenate(
            [64 * h + np.concatenate([np.arange(0, 64, 2),
                                      np.arange(1, 64, 2)])
             for h in heads])
        vcols = np.concatenate([64 * h + np.arange(64) for h in heads])
        xT = x[b].T  # [D, S] float32
        xt_t = np.ascontiguousarray(
            xT.reshape(NDB, 128, NSC, SC).transpose(1, 2, 0, 3))
        im = {
            "xt": xt_t.astype(NP_BF16),
            "xt8": (xt_t * SX).astype(NP_F8),
            "wq": (tile_rows(Wq[:, perm], NDB) * SW).astype(NP_F8),
            "wk": (tile_rows(Wk[:, perm], NDB) * SW).astype(NP_F8),
            "wv": tile_rows(Wv[:, vcols], NDB).astype(NP_BF16),
            "wo": tile_rows(Wo[vcols, :], 2).astype(NP_BF16),
            "cosT": cosT, "sinT": sinT,
            "pmat": pmat.astype(NP_BF16),
            "ones": ones, "masktri": mtri2,
        }
        in_maps.append(im)
    return in_maps


_NC_CACHE = {}


def get_program(niter=1, phases="paw", unroll=1):
    key = (niter, phases, unroll)
    if key not in _NC_CACHE:
        _NC_CACHE[key] = build_program(niter, phases, unroll=unroll)
    return _NC_CACHE[key]


def run_on_cores(nc, in_maps, **kw):
    return run_bass_kernel_spmd(nc, in_maps, list(range(N_CORES)), **kw)


def unpack_out(arr):
    """[128, NSC, NDB, SC] -> [D, S] float32 partial."""
    return np.ascontiguousarray(
        np.asarray(arr, np.float32).transpose(2, 0, 1, 3).reshape(D, S))


def kernel(x, pos_cos, pos_sin, Wq, Wk, Wv, Wo, bo):
    nc = get_program(1)
    in_maps = prep_inputs(x, pos_cos, pos_sin, Wq, Wk, Wv, Wo)
    res = run_on_cores(nc, in_maps)
    out = np.empty((B, S, D), np.float32)
    for b in range(B):
        acc = unpack_out(res.results[4 * b]["outp"]).astype(np.float64)
        for g in range(1, 4):
            acc += unpack_out(res.results[4 * b + g]["outp"])
        out[b] = (acc.T + np.asarray(bo, np.float64)[None, :]).astype(np.float32)
    return out


# revision 42
# speedup vs baseline: 1.3883x; 1.3883x over previous
"""Trainium2 Bass kernel for nn_MultiHeadAttention (B=2, S=2048, D=1024, H=16).

Sharding: 8 cores = 2 batch groups x 4 head-groups (4 heads/core).
Per core: QKV projections (fp8 DoubleRow for Q/K, bf16 for V), RoPE via
batched DVE mults + a PE permutation matmul that reuses the projection
psum in place (merged rotate-half layout: each score block is a single
64-contraction matmul), causal attention with explicitly row-tiled score
matmuls (two 64-row PE tiles run concurrently), bf16 PV with fused rowsum
via [V|ones] stationary, bf16 output projection partials; host sums the
4 partials per batch and adds the bias.

Self-contained: hardcodes shapes; only imports concourse/numpy.
"""
import numpy as np
from contextlib import ExitStack

import concourse.bass as bass
import concourse.bacc as bacc
import concourse.mybir as mybir
import concourse.tile as tile
from concourse.bass_utils import run_bass_kernel_spmd

F32 = mybir.dt.float32
BF16 = mybir.dt.bfloat16
I16 = mybir.dt.int16
AF = mybir.ActivationFunctionType
OP = mybir.AluOpType
NP_BF16 = mybir.dt.np(mybir.dt.bfloat16)

# Schraudolph-style exp on DVE: bf16 bits of exp(s/8) ~= round(EK*s + EB)
# (bf16 exponent step = 128 mantissa codes; c=0.0573 balances the error)
EK = 128.0 * 1.4426950408889634 * 0.125
EB = 128.0 * (127.0 - 0.0573)
DVE_EXP_MOD = 5   # every Nth (jb, hp) slot's exp runs on DVE, 0 = off
PV_PENDING = 4    # PV deferral depth (slots)
PT_BUFS = 6       # pt tile double-buffer depth

F8 = mybir.dt.float8e4
NP_F8 = mybir.dt.np(mybir.dt.float8e4)
SX, SW = 32.0, 512.0      # fp8 pre-scales for x and Wq/Wk (powers of 2)

B, S, D, H = 2, 2048, 1024, 16
HD = 64          # head dim
HPC = 4          # heads per core
N_CORES = 8
SC = 512         # s-chunk for projections / i-tile width
NSC = S // SC    # 4
NDB = D // 128   # 8 d-blocks
NJB = S // 128   # 16 j-blocks
DLOC = HPC * HD  # 256 local channels


def build_program(niter=1, phases='paw', dbg=False, unroll=1):
    """Build + compile the per-core SPMD program. niter>1 wraps the whole
    kernel in a device-side loop (for timing)."""
    nc = bacc.Bacc("TRN2", target_bir_lowering=False, debug=False,
                   num_devices=N_CORES)

    # host-pre-tiled layouts: flat per-partition strips -> few descriptors
    d_xt = nc.dram_tensor("xt", [128, NSC, NDB, SC], BF16,
                          kind="ExternalInput").ap()
    d_xt8 = nc.dram_tensor("xt8", [128, NSC, NDB, SC], F8,
                           kind="ExternalInput").ap()
    d_wq = nc.dram_tensor("wq", [128, NDB, DLOC], F8,
                          kind="ExternalInput").ap()
    d_wk = nc.dram_tensor("wk", [128, NDB, DLOC], F8,
                          kind="ExternalInput").ap()
    d_wv = nc.dram_tensor("wv", [128, NDB, DLOC], BF16,
                          kind="ExternalInput").ap()
    d_wo = nc.dram_tensor("wo", [128, 2, D], BF16, kind="ExternalInput").ap()
    d_cos = nc.dram_tensor("cosT", [128, S], BF16, kind="ExternalInput").ap()
    d_sin = nc.dram_tensor("sinT", [128, S], BF16, kind="ExternalInput").ap()
    d_pm = nc.dram_tensor("pmat", [128, 128], BF16, kind="ExternalInput").ap()
    d_ones = nc.dram_tensor("ones", [128, NJB * HPC * 64], BF16,
                            kind="ExternalInput").ap()
    d_mask = nc.dram_tensor("masktri", [128, 2, 128], BF16,
                            kind="ExternalInput").ap()
    d_out = nc.dram_tensor("outp", [128, NSC, NDB, SC], BF16,
                           kind="ExternalOutput").ap()

    with tile.TileContext(nc) as tc:
        ctx = ExitStack()
        sb = ctx.enter_context(tc.tile_pool(name="sb", bufs=1))
        ps = ctx.enter_context(tc.tile_pool(name="ps", bufs=1, space="PSUM"))

        # persistent inputs (loaded once, outside the timing loop)
        wq = sb.tile([128, NDB, DLOC], F8, tag="wq")
        wk = sb.tile([128, NDB, DLOC], F8, tag="wk")
        wv = sb.tile([128, NDB, DLOC], BF16, tag="wv")
        wo = sb.tile([128, 2, D], BF16, tag="wo")
        cosT = sb.tile([128, S], BF16, tag="cs0")
        sinT = sb.tile([128, S], BF16, tag="cs1")
        pmat = sb.tile([128, 128], BF16, tag="pm")
        mtri2 = sb.tile([128, 2, 128], BF16, tag="mt2")
        # ordered by first use: Q/K weights + rope tables first (the QK
        # projection is the first PE work), V/ones/wo later
        nc.gpsimd.dma_start(wq[:], d_wq[:])
        nc.gpsimd.dma_start(wk[:], d_wk[:])
        nc.scalar.dma_start(cosT[:], d_cos[:])
        nc.scalar.dma_start(sinT[:], d_sin[:])
        nc.scalar.dma_start(pmat[:], d_pm[:])
        nc.scalar.dma_start(mtri2[:], d_mask[:])
        nc.scalar.dma_start(wv[:], d_wv[:])
        # V' = [j, jb, head, V(64)|ones(64)] -- persistent; ones loaded once
        vp = sb.tile([128, NJB, HPC, 128], BF16, tag="vp")
        nc.scalar.dma_start(
            vp[:, :, :, 64:128],
            d_ones[:].rearrange("p (j h c) -> p j h c", j=NJB, h=HPC))
        nc.gpsimd.dma_start(wo[:], d_wo[:])

        # rope'd Q/K in merged rotate-half layout:
        # qf[p, hp, s]: half hp holds heads (2hp, 2hp+1); within a half,
        # head block rows = [32 rot-evens | 32 rot-odds].
        qf = sb.tile([128, 2, S], BF16, tag="qf")
        kf = sb.tile([128, 2, S], BF16, tag="kf")
        cxt = sb.tile([128, 2, S], BF16, tag="cx")  # ctxT, halves=head pairs
        # dedicated chunk-0 x tiles: their load is prefetched from the tail
        # of the previous loop iteration (chunks 1-3 double-buffer on a tag)
        xt0 = sb.tile([128, NDB, SC], BF16, tag="xt0")
        xt80 = sb.tile([128, NDB, SC], F8, tag="xt80", name="xt80")

        def load_xt0():
            nc.sync.dma_start(xt80[:], d_xt8[:, 0])
            nc.sync.dma_start(xt0[:], d_xt[:, 0])

        def body(_i=None, in_loop=False, only_proj0=False, carry=False):

            xt_tiles = {}
            xt8_tiles = {}
            if in_loop:
                # chunk 0 was loaded by the previous iteration (or prologue)
                xt_tiles[0] = xt0
                xt8_tiles[0] = xt80

            def xt_load_gen(sc):
                if sc == 0:
                    xt, xt8 = xt0, xt80
                else:
                    xt = sb.tile([128, NDB, SC], BF16, tag="xt", bufs=2,
                                 name="xt")
                    xt8 = sb.tile([128, NDB, SC], F8, tag="xt8", bufs=2,
                                  name="xt8")
                nc.sync.dma_start(xt8[:], d_xt8[:, sc])
                nc.sync.dma_start(xt[:], d_xt[:, sc])
                xt_tiles[sc] = xt
                xt8_tiles[sc] = xt8
                yield

            def proj_qk_gen(sc):
                """Q/K projection + RoPE for chunk `sc` as PE-work quanta.

                pp rides the shared 'st' rotation (with the score psum)
                so quanta can interleave into the attention loop; the swap
                matmul writes back into pp's banks once the rope mults
                have consumed them.
                """
                s0 = sc * SC
                if sc not in xt_tiles:
                    yield from xt_load_gen(sc)
                xt8 = xt8_tiles[sc]
                csb = cosT[:, s0:s0 + SC].unsqueeze(1).broadcast_to(
                    [128, 2, SC])
                snb = sinT[:, s0:s0 + SC].unsqueeze(1).broadcast_to(
                    [128, 2, SC])

                def proj_mm(w, pp, hp):
                    for t in range(NDB // 2):
                        nc.tensor.matmul(
                            pp[:, hp, :],
                            w[:, 2 * t:2 * t + 2, 128 * hp:128 * hp + 128],
                            xt8[:, 2 * t:2 * t + 2, :],
                            start=(t == 0), stop=(t == NDB // 2 - 1),
                            perf_mode=mybir.MatmulPerfMode.DoubleRow)

                def rope_mults(pp, t1, t2):
                    nc.vector.tensor_tensor(t1[:], pp[:], csb, OP.mult)
                    nc.vector.tensor_tensor(t2[:], pp[:], snb, OP.mult)

                def swap_mm(pp, t2):
                    # pp <- pmat @ t2 (reuses pp's own banks)
                    for hp in range(2):
                        nc.tensor.matmul(pp[:, hp, :], pmat[:],
                                         t2[:, hp, :], start=True, stop=True)

                def rope_add(dst, t1, pp):
                    nc.vector.tensor_tensor(dst[:, :, s0:s0 + SC],
                                            t1[:], pp[:], OP.add)

                def mk_tiles():
                    pp = ps.tile([128, 2, SC], F32, tag="st", bufs=2,
                                 name="pp")
                    t1 = sb.tile([128, 2, SC], BF16, tag="t1", bufs=2,
                                 name="t1")
                    t2 = sb.tile([128, 2, SC], BF16, tag="t2", bufs=2,
                                 name="t2")
                    return pp, t1, t2

                # software-pipelined: each swap runs >=2 slots after its
                # rope mults so the in-order PE never waits on the DVE
                ppq, t1q, t2q = mk_tiles()
                proj_mm(wq, ppq, 0)
                yield
                proj_mm(wq, ppq, 1)
                rope_mults(ppq, t1q, t2q)
                yield
                ppk, t1k, t2k = mk_tiles()
                proj_mm(wk, ppk, 0)
                yield
                swap_mm(ppq, t2q)
                yield
                proj_mm(wk, ppk, 1)
                rope_mults(ppk, t1k, t2k)
                rope_add(qf, t1q, ppq)
                yield
                yield
                swap_mm(ppk, t2k)
                yield
                rope_add(kf, t1k, ppk)
                yield

            def proj_v_gen(sc):
                """V projection for chunk `sc`; vps shares the pv banks so
                it may only run while no pv accumulation is live."""
                xt = xt_tiles[sc]
                for vb in range(2):
                    jb0 = sc * (SC // 128) + 2 * vb
                    vps = ps.tile([128, 2, DLOC], F32, tag=f"pv{2 * vb}",
                                  name="vps")
                    for k in range(2):
                        sb4 = 2 * vb + k
                        for db in range(NDB):
                            nc.tensor.matmul(
                                vps[:, k, :],
                                xt[:, db, 128 * sb4:128 * sb4 + 128],
                                wv[:, db, :],
                                start=(db == 0), stop=(db == NDB - 1))
                    nc.vector.tensor_copy(
                        vp[:, jb0:jb0 + 2, :, 0:64],
                        vps[:].rearrange("p b (h c) -> p b h c", h=HPC))
                    yield

            def wo_gen(it):
                """Output projection for i-tile `it` as PE-work quanta."""
                i0 = it * SC
                ost = sb.tile([128, NDB, SC], BF16, tag="ost", bufs=2,
                              name="ost")
                for dt in range(NDB):
                    ops_ = ps.tile([128, SC], F32, tag=f"pv{2 * (dt % 2)}",
                                   name="ops")
                    for dcb in range(2):
                        nc.tensor.matmul(
                            ops_[:], wo[:, dcb, dt * 128:dt * 128 + 128],
                            cxt[:, dcb, i0:i0 + SC],
                            start=(dcb == 0), stop=(dcb == 1))
                    if dt % 2 == 0:
                        nc.scalar.activation(ost[:, dt, :], ops_[:], AF.Copy)
                    else:
                        nc.vector.tensor_copy(ost[:, dt, :], ops_[:])
                    # stream the output out per d-block so the final DMA
                    # tail after the last i-tile is short
                    nc.sync.dma_start(d_out[:, it, dt], ost[:, dt, :])
                    yield

            def drain(gens):
                for g in gens:
                    for _ in g:
                        pass

            def attn_itile(it, fill=(), pre=(), post=()):
                """Attention i-tile; drains one filler quantum per
                (jb, head-pair) slot so the PE never idles on exp.
                `pre` generators drain fully at entry (before the pv
                accumulators are allocated); `post` after normalize."""
                drain(pre)

                def filler():
                    for g in fill:
                        yield from g
                f = filler()
                i0 = it * SC
                njb = i0 // 128 + 4
                # pvs2[g][:, u, :] accumulates head 2g+u; [64:128] = rowsum
                pvs2 = [ps.tile([128, 2, SC], F32, tag=f"pv{2 * g}",
                                name=f"pvt{g}") for g in range(2)]
                pending = []
                escale = 0.125
                slot = 0
                for jb in range(njb):
                    j0 = jb * 128
                    off = max(0, j0 - i0)
                    for hp in range(2):  # head pairs (0,1) and (2,3)
                        ss = ps.tile([128, 2, SC], F32, tag="st", bufs=2,
                                     name="ssjb")
                        for hh in range(2):
                            r0 = 64 * hh
                            nc.tensor.matmul(
                                ss[:, hh, off:SC],
                                kf[r0:r0 + 64, hp, j0:j0 + 128],
                                qf[r0:r0 + 64, hp, i0 + off:i0 + SC],
                                start=True, stop=True,
                                tile_position=(r0, 0))
                        pt = sb.tile([128, 2, SC], BF16, tag="pt",
                                     bufs=PT_BUFS, name="pt")
                        slot += 1
                        if DVE_EXP_MOD and slot % DVE_EXP_MOD == 0:
                            # approximate exp on DVE (bf16 bit trick) to
                            # take load off ACT, the attention-phase pacer
                            nc.vector.tensor_scalar(
                                pt[:, :, off:SC].bitcast(I16),
                                ss[:, :, off:SC], EK, EB, OP.mult, OP.add)
                        else:
                            nc.scalar.activation(pt[:, :, off:SC],
                                                 ss[:, :, off:SC],
                                                 AF.Exp, scale=escale)
                        if j0 >= i0:
                            # diagonal: mask both heads' [128,128] windows
                            nc.vector.tensor_tensor(
                                pt[:, :, off:off + 128],
                                pt[:, :, off:off + 128], mtri2[:], OP.mult)

                        def mk_pv(jb=jb, hp=hp, pt=pt, off=off):
                            for hh in range(2):
                                h = 2 * hp + hh
                                nc.tensor.matmul(
                                    pvs2[h // 2][:, h % 2, off:SC],
                                    vp[:, jb, h, :], pt[:, hh, off:SC],
                                    start=(jb == 0),
                                    stop=(jb == njb - 1))
                        pending.append(mk_pv)
                        # previous block's PV after this block's scores so
                        # the PE stream never stalls on the current exp
                        while len(pending) > PV_PENDING:
                            pending.pop(0)()
                        next(f, None)
                for fn in pending:
                    fn()
                # normalize: ctx = pv[0:64] / r (r = pv[64:128])
                for g in range(2):
                    # reciprocal_approx_fast cannot read PSUM on HW --
                    # bounce the rowsums through SBUF first
                    rt = sb.tile([64, 2, SC], F32, tag="rt", bufs=2,
                                 name="rt")
                    nc.vector.tensor_copy(rt[:], pvs2[g][64:128, :, :])
                    rr = sb.tile([64, 2, SC], F32, tag="rr", bufs=2,
                                 name="rr")
                    nc.vector.reciprocal_approx_fast(rr[:], rt[:])
                    for u in range(2):
                        h = 2 * g + u
                        nc.vector.tensor_tensor(
                            cxt[64 * (h % 2):64 * (h % 2) + 64, h // 2,
                                i0:i0 + SC],
                            pvs2[g][0:64, u, :], rr[:, u, :], OP.mult)
                drain([f])
                drain(post)

            if only_proj0:
                # prologue for the carried loop: chunk-0 proj only
                drain([proj_qk_gen(0), proj_v_gen(0)])
                return

            w_ = "w" in phases
            if "p" in phases and "a" in phases:
                if not carry:
                    drain([proj_qk_gen(0), proj_v_gen(0)])
                attn_itile(0, fill=[proj_qk_gen(1)], post=[proj_v_gen(1)])
                attn_itile(1, fill=[proj_qk_gen(2)], post=[proj_v_gen(2)],
                           pre=[wo_gen(0)] if w_ else ())
                attn_itile(2, fill=[proj_qk_gen(3)], post=[proj_v_gen(3)],
                           pre=[wo_gen(1)] if w_ else ())
                # carried loop: the NEXT iteration's chunk-0 reload + Q/K
                # projection fill attn3's exp-paced PE idle slots; its V
                # projection runs as post (pv banks)
                attn_itile(3, pre=[wo_gen(2)] if w_ else (),
                           fill=([xt_load_gen(0), proj_qk_gen(0)]
                                 if carry else ()),
                           post=([proj_v_gen(0)] if carry else ()))
                if w_:
                    drain([wo_gen(3)])
            elif "p" in phases:
                for sc_ in range(NSC):
                    drain([proj_qk_gen(sc_), proj_v_gen(sc_)])
            elif "a" in phases:
                for it_ in range(NSC):
                    attn_itile(it_)
                if w_:
                    for it_ in range(NSC):
                        drain([wo_gen(it_)])
            elif w_:
                for it_ in range(NSC):
                    drain([wo_gen(it_)])

        carried = "p" in phases and "a" in phases
        if unroll > 1:
            # unrolled steady-state emulation (no hw loop): body n times
            # with the same chunk-0 carry structure as the real loop
            load_xt0()
            if carried:
                body(in_loop=True, only_proj0=True)
            for _ in range(unroll):
                body(in_loop=carried, carry=carried)
        elif niter == 1:
            body()
        else:
            if "p" not in phases:
                # populate qf/kf/vp/cxt once so the timed loop has real data
                saved = phases
                phases = "pa"
                body()
                phases = saved
            load_xt0()  # prologue; in-loop reloads happen at attn3
            if carried:
                body(in_loop=True, only_proj0=True)
            with tc.For_i(0, niter, 1, staggered_reset=True) as i:
                body(i, in_loop=carried, carry=carried)
        ctx.close()
    nc.compile()
    return nc


def prep_inputs(x, pos_cos, pos_sin, Wq, Wk, Wv, Wo):
    """Host-side prep: per-core input dicts (pre-tiled)."""
    # fp8 scale folded out of the rope tables (tiny activation scales
    # underflow on HW, so fold into cos/sin instead)
    tscale = 1.0 / (SX * SW)
    cosT = np.ascontiguousarray(
        (np.tile(np.asarray(pos_cos, np.float32).T, (4, 1))
         * np.float32(tscale)).astype(NP_BF16))
    sinT = np.ascontiguousarray(
        (np.tile(np.asarray(pos_sin, np.float32).T, (4, 1))
         * np.float32(tscale)).astype(NP_BF16))
    ones = np.ones((128, NJB * HPC * 64), NP_BF16)
    # mask[j, i] keeps i >= j within the diagonal 128x128 window
    mtri = np.triu(np.ones((128, 128), np.float32)).astype(NP_BF16)
    mtri2 = np.ascontiguousarray(
        np.broadcast_to(mtri[:, None, :], (128, 2, 128)))
    # permutation matmul: out[p] = -t2[p+32] (even block) / +t2[p-32] (odd)
    pmat = np.zeros((128, 128), np.float32)
    for p in range(128):
        if (p % 64) < 32:
            pmat[p + 32, p] = -1.0
        else:
            pmat[p - 32, p] = 1.0

    def tile_rows(a, nb):
        # [nb*128, C] -> [128, nb, C]
        return np.ascontiguousarray(
            a.reshape(nb, 128, a.shape[1]).transpose(1, 0, 2))

    x = np.asarray(x, np.float32)
    Wq, Wk, Wv, Wo = (np.asarray(w, np.float32) for w in (Wq, Wk, Wv, Wo))
    in_maps = []
    for c in range(N_CORES):
        b, g = c // 4, c % 4
        heads = [4 * g + h for h in range(HPC)]
        # rotate-half head-major: per head [evens(32) | odds(32)]
        perm = np.concatenate(
            [64 * h + np.concatenate([np.arange(0, 64, 2),
                                      np.arange(1, 64, 2)])
             for h in heads])
        vcols = np.concatenate([64 * h + np.arange(64) for h in heads])
        xT = x[b].T  # [D, S] float32
        xt_t = np.ascontiguousarray(
            xT.reshape(NDB, 128, NSC, SC).transpose(1, 2, 0, 3))
        im = {
            "xt": xt_t.astype(NP_BF16),
            "xt8": (xt_t * SX).astype(NP_F8),
            "wq": (tile_rows(Wq[:, perm], NDB) * SW).astype(NP_F8),
            "wk": (tile_rows(Wk[:, perm], NDB) * SW).astype(NP_F8),
            "wv": tile_rows(Wv[:, vcols], NDB).astype(NP_BF16),
            "wo": tile_rows(Wo[vcols, :], 2).astype(NP_BF16),
            "cosT": cosT, "sinT": sinT,
            "pmat": pmat.astype(NP_BF16),
            "ones": ones, "masktri": mtri2,
        }
        in_maps.append(im)
    return in_maps


_NC_CACHE = {}


def get_program(niter=1, phases="paw", unroll=1):
    key = (niter, phases, unroll)
    if key not in _NC_CACHE:
        _NC_CACHE[key] = build_program(niter, phases, unroll=unroll)
    return _NC_CACHE[key]


def run_on_cores(nc, in_maps, **kw):
    return run_bass_kernel_spmd(nc, in_maps, list(range(N_CORES)), **kw)


def unpack_out(arr):
    """[128, NSC, NDB, SC] -> [D, S] float32 partial."""
    return np.ascontiguousarray(
        np.asarray(arr, np.float32).transpose(2, 0, 1, 3).reshape(D, S))


def kernel(x, pos_cos, pos_sin, Wq, Wk, Wv, Wo, bo):
    nc = get_program(1)
    in_maps = prep_inputs(x, pos_cos, pos_sin, Wq, Wk, Wv, Wo)
    res = run_on_cores(nc, in_maps)
    out = np.empty((B, S, D), np.float32)
    for b in range(B):
        acc = unpack_out(res.results[4 * b]["outp"]).astype(np.float64)
        for g in range(1, 4):
            acc += unpack_out(res.results[4 * b + g]["outp"])
        out[b] = (acc.T + np.asarray(bo, np.float64)[None, :]).astype(np.float32)
    return out
